# revision 25
# baseline (speedup 1.0000x reference)
"""Trainium2 Bass kernel for the kinematic bicycle-model rollout.

Strategy (v3, ~119us vs 165us baseline)
---------------------------------------
The rollout is affine in the start state with batch-independent
coefficients; the host precomputes (f64) the [H] vectors
    c[t]  = sum_{i<t} DT*MAX_ACC*clip(a_i)          (speed_t = s0 + c_t)
    A[t]  = sum_{i<t} k_i,  Bv[t] = sum_{i<t} c_i*k_i,
            k_i = tan(clip(s_i))/WHEELBASE*DT       (yaw_t = yaw0 + s0*A_t + Bv_t)
leaving x/y on-chip:  x_t = x0 + sum_{i<t} (DT*c_i + DT*s0) * cos(yaw_i).

Per core (data-parallel over batch, 1024 rollouts), batch on the 128 SBUF
partitions (8 tiles), time on the free dim.  The two DVE prefix scans are
the hard floor (~71us/core, ~2.2ns/col, dtype-independent), so everything
else is pushed off DVE:

  PE   f16 matmuls into PSUM (f16 streams ~1.2ns/col incl fp32 accum):
         nw1 = round(u) - u + yaw0'   via K=7 rows
               [yaw0', s0*A', Bv', +MAGIC, -MAGIC, -s0*A', -Bv']
               with u = yaw/(2pi); MAGIC=1.5*2^23 enters as the f16-exact
               product 1024*12288, and PSUM accumulates rows in order in
               fp32, so fl(u+MAGIC)-MAGIC realizes round(u) exactly
         nw2 = round(u+0.25) - u + yaw0'  (K=8, extra +0.25 row; the 0.25
               phase turns Sin into cos at exactly +-pi arg range)
         yawp = s0*A + Bv  (radians, K=2, 1024-wide PSUM pairs)
  ACT  sin16 = Sin(-2pi*nw1 + 2pi*yaw0'_f16)   [bias uses the f16-rounded
       cos16 = Sin(-2pi*nw2 + 2pi*yaw0'_f16 + pi/2)   yaw0' so the matmul
       term cancels exactly; args land in +-pi, LUT valid to ~+-3.2]
       yaw16 = yawp + yaw0;  m16 = cdt16 + DT*s0;  speed16 = cdt16/DT + s0
  DVE  vx16 = m16*cos16, vy16 = m16*sin16 (tensor_tensor f16, 2 elem/cyc)
       x/y prefix scans (f16 in/out, fp32 internal state, chained per
       1024-col chunk through the f16 boundary column)
GPSIMD is deliberately idle: Pool cannot run TensorScalarPtr ops or touch
PSUM on V3, and its tensor_tensor co-running with DVE inflates both ~60%
via SBUF port contention (measured 165us -> 179us).

All outputs leave the device as f16 (rel gate is 2e-2, f16 gives ~5e-4),
halving DMA traffic to 16.8MB/core; the host casts to f32, transposes to
the reference [H, B] layout, and overwrites row 0 with the exact f32
start state (device leaves x/y col 0 unwritten).
"""

import math
import sys

sys.path.insert(0, "/opt/trn_rl_repo")

import numpy as np

import concourse.bacc as bacc
import concourse.mybir as mybir
import concourse.tile as tile
from concourse.bass_utils import run_bass_kernel_spmd

# Model constants (match the reference nn.Module)
H = 2048
B = 8192
NCORES = 8
BL = B // NCORES          # batch per core
P = 128                   # SBUF partitions
NPT = BL // P             # batch tiles per core
DT = 0.05
WHEELBASE = 2.5
MAX_STEER = 0.5
MAX_ACC = 5000.0 / 1000.0

TWO_PI = 2.0 * math.pi
INV_2PI = 1.0 / TWO_PI
HALF_PI = 0.5 * math.pi
MAG_L = 1024.0            # MAGIC = MAG_L * MAG_R = 1.5*2^23, f16-exact factors
MAG_R = 12288.0

F32 = mybir.dt.float32
F16 = mybir.dt.float16
AFT = mybir.ActivationFunctionType
ALU = mybir.AluOpType

K1 = 7                    # nw1 matmul rows
K2 = 8                    # nw2 matmul rows
HH = 512                  # column segment = one PSUM bank of f32

_CACHE = {}


def _build():
    nc = bacc.Bacc("TRN2", target_bir_lowering=False, debug=False)

    rhs1 = nc.declare_dram_parameter("rhs1", [K1, H], F16, isOutput=False)
    rhs2 = nc.declare_dram_parameter("rhs2", [K2, H], F16, isOutput=False)
    rhsy = nc.declare_dram_parameter("rhsy", [2, H], F16, isOutput=False)
    lhs1 = nc.declare_dram_parameter("lhs1", [K1, BL], F16, isOutput=False)
    lhs2 = nc.declare_dram_parameter("lhs2", [K2, BL], F16, isOutput=False)
    lhsy = nc.declare_dram_parameter("lhsy", [2, BL], F16, isOutput=False)
    cdt16 = nc.declare_dram_parameter("cdt16", [H], F16, isOutput=False)
    # per-batch f32 columns: [sinb, cosb, yaw0, sdt0, x0, y0, s0]
    cols = nc.declare_dram_parameter("cols", [BL, 7], F32, isOutput=False)
    ox = nc.declare_dram_parameter("ox", [BL, H], F16, isOutput=True)
    oy = nc.declare_dram_parameter("oy", [BL, H], F16, isOutput=True)
    oyaw = nc.declare_dram_parameter("oyaw", [BL, H], F16, isOutput=True)
    ospeed = nc.declare_dram_parameter("ospeed", [BL, H], F16, isOutput=True)

    with tile.TileContext(nc) as tc:
        with (
            tc.tile_pool(name="const", bufs=1) as constp,
            tc.tile_pool(name="io", bufs=3) as iop,
            tc.tile_pool(name="mid", bufs=4) as midp,
            tc.tile_pool(name="psum", bufs=2, space="PSUM") as psp,
        ):
            cdt_bc = constp.tile([P, H], F16)
            nc.sync.dma_start(out=cdt_bc[:], in_=cdt16[None, :].to_broadcast((P, H)))
            r1_sb = constp.tile([K1, H], F16)
            nc.sync.dma_start(out=r1_sb[:], in_=rhs1[:])
            r2_sb = constp.tile([K2, H], F16)
            nc.sync.dma_start(out=r2_sb[:], in_=rhs2[:])
            ry_sb = constp.tile([2, H], F16)
            nc.sync.dma_start(out=ry_sb[:], in_=rhsy[:])
            l1_sb = constp.tile([K1, BL], F16)
            nc.sync.dma_start(out=l1_sb[:], in_=lhs1[:])
            l2_sb = constp.tile([K2, BL], F16)
            nc.sync.dma_start(out=l2_sb[:], in_=lhs2[:])
            ly_sb = constp.tile([2, BL], F16)
            nc.sync.dma_start(out=ly_sb[:], in_=lhsy[:])

            NSEG = H // HH
            for pt in range(NPT):
                sl = slice(pt * P, (pt + 1) * P)
                colt = iop.tile([P, 7], F32, tag="colt")
                nc.sync.dma_start(out=colt[:], in_=cols[sl, :])
                sinb_c = colt[:, 0:1]
                cosb_c = colt[:, 1:2]
                yaw0_c = colt[:, 2:3]
                sdt0_c = colt[:, 3:4]
                x0_c = colt[:, 4:5]
                y0_c = colt[:, 5:6]
                s0_c = colt[:, 6:7]

                # col 0 of x/y is left unwritten on device; the host
                # overwrites output row 0 with the exact f32 start state
                x_sb = iop.tile([P, H], F16, tag="x")
                y_sb = iop.tile([P, H], F16, tag="y")

                # m16/speed16 first: they only need cdt_bc+colt, so ACT can
                # emit them before the first matmul results land
                m16 = midp.tile([P, H], F16, tag="m16")
                nc.scalar.activation(out=m16[:], in_=cdt_bc[:], func=AFT.Identity,
                                     bias=sdt0_c, scale=1.0)
                speed16 = midp.tile([P, H], F16, tag="sp16")
                nc.scalar.activation(out=speed16[:], in_=cdt_bc[:],
                                     func=AFT.Identity, bias=s0_c,
                                     scale=1.0 / DT)

                sin_t = midp.tile([P, H], F16, tag="sin")
                cos_t = midp.tile([P, H], F16, tag="cos")
                yaw16 = midp.tile([P, H], F16, tag="yaw16")
                for hf in range(NSEG):
                    cs = slice(hf * HH, (hf + 1) * HH)
                    ps1 = psp.tile([P, HH], F32, tag="ps1")
                    nc.tensor.matmul(ps1[:], l1_sb[:, sl], r1_sb[:, cs])
                    ps2 = psp.tile([P, HH], F32, tag="ps2")
                    nc.tensor.matmul(ps2[:], l2_sb[:, sl], r2_sb[:, cs])
                    if hf % 2 == 0:
                        psy = psp.tile([P, 2 * HH], F32, tag="psy")
                        nc.tensor.matmul(psy[:, 0:HH], ly_sb[:, sl],
                                         ry_sb[:, cs])
                        nc.tensor.matmul(psy[:, HH:2 * HH], ly_sb[:, sl],
                                         ry_sb[:, cs.stop:cs.stop + HH])

                    nc.scalar.activation(out=sin_t[:, cs], in_=ps1[:],
                                         func=AFT.Sin, bias=sinb_c,
                                         scale=-TWO_PI)
                    nc.scalar.activation(out=cos_t[:, cs], in_=ps2[:],
                                         func=AFT.Sin, bias=cosb_c,
                                         scale=-TWO_PI)
                    if hf % 2 == 1:
                        ys = slice((hf - 1) * HH, (hf + 1) * HH)
                        nc.scalar.activation(out=yaw16[:, ys], in_=psy[:],
                                             func=AFT.Identity, bias=yaw0_c,
                                             scale=1.0)

                nc.sync.dma_start(out=oyaw[sl, :], in_=yaw16[:])
                nc.sync.dma_start(out=ospeed[sl, :], in_=speed16[:])

                vx = midp.tile([P, H], F16, tag="vx")
                vy = midp.tile([P, H], F16, tag="vy")
                CHUNKS = [(0, 1024), (1024, 1024)]
                for ci, (lo, sz) in enumerate(CHUNKS):
                    bs = slice(lo, lo + sz)
                    nd = sz if ci < len(CHUNKS) - 1 else sz - 1
                    nc.vector.tensor_tensor(out=vx[:, bs], in0=m16[:, bs],
                                            in1=cos_t[:, bs], op=ALU.mult)
                    nc.vector.tensor_tensor_scan(
                        out=x_sb[:, lo + 1:lo + 1 + nd], data0=vx[:, lo:lo + nd],
                        data1=vx[:, lo:lo + nd],
                        initial=(x0_c if lo == 0 else x_sb[:, lo:lo + 1]),
                        op0=ALU.add, op1=ALU.bypass,
                    )
                    nc.vector.tensor_tensor(out=vy[:, bs], in0=m16[:, bs],
                                            in1=sin_t[:, bs], op=ALU.mult)
                    nc.vector.tensor_tensor_scan(
                        out=y_sb[:, lo + 1:lo + 1 + nd], data0=vy[:, lo:lo + nd],
                        data1=vy[:, lo:lo + nd],
                        initial=(y0_c if lo == 0 else y_sb[:, lo:lo + 1]),
                        op0=ALU.add, op1=ALU.bypass,
                    )
                    nc.sync.dma_start(out=ox[sl, bs], in_=x_sb[:, bs])
                    nc.sync.dma_start(out=oy[sl, bs], in_=y_sb[:, bs])

    nc.finalize()
    return nc


def _host_precompute(accel, steering):
    a = np.clip(accel.astype(np.float64), -1.0, 1.0)
    dv = DT * MAX_ACC * a
    c = np.concatenate([[0.0], np.cumsum(dv)[: H - 1]])
    st = np.clip(steering.astype(np.float64), -MAX_STEER, MAX_STEER)
    k = np.tan(st) / WHEELBASE * DT
    A = np.concatenate([[0.0], np.cumsum(k)[: H - 1]])
    Bv = np.concatenate([[0.0], np.cumsum(c * k)[: H - 1]])
    return A, Bv, c


def _build_mats(A, Bv, c, start_yaw, start_speed):
    """Host-side f16 matmul operands + f32 bias columns (per core slice)."""
    Ap = (A * INV_2PI).astype(np.float16)
    Bp = (Bv * INV_2PI).astype(np.float16)
    ones_h = np.ones(H, np.float16)
    rhs1 = np.stack([ones_h, Ap, Bp, ones_h * MAG_R, ones_h * MAG_R, Ap, Bp])
    rhs2 = np.stack([ones_h, Ap, Bp, ones_h * 0.25, ones_h * MAG_R,
                     ones_h * MAG_R, Ap, Bp])
    rhsy = np.stack([A.astype(np.float16), Bv.astype(np.float16)])
    cdt16 = (DT * c).astype(np.float16)

    s0_16 = start_speed.astype(np.float16)
    yawp_16 = (start_yaw.astype(np.float64) * INV_2PI).astype(np.float16)
    ones_b = np.ones(BL, np.float16)
    lhs1 = np.stack([yawp_16, s0_16, ones_b, ones_b * MAG_L, -ones_b * MAG_L,
                     -s0_16, -ones_b])
    lhs2 = np.stack([yawp_16, s0_16, ones_b, ones_b, ones_b * MAG_L,
                     -ones_b * MAG_L, -s0_16, -ones_b])
    lhsy = np.stack([s0_16, ones_b])
    return rhs1, rhs2, rhsy, cdt16, lhs1, lhs2, lhsy, s0_16, yawp_16


def _install_ntff_shim():
    """antenv.axon_hooks is absent in this image; recreate it so
    run_bass_kernel_spmd(trace=True) can reach the axon NTFF profiler."""
    import types

    import antenv

    if hasattr(antenv, "axon_hooks"):
        return
    mod = types.ModuleType("antenv.axon_hooks")
    holder = [None]
    mod.set_axon_ntff_profile_hook = lambda h: holder.__setitem__(0, h)
    mod.get_axon_ntff_profile_hook = lambda: holder[0]
    sys.modules["antenv.axon_hooks"] = mod
    antenv.axon_hooks = mod
    from trn_agent_boot.trn_boot import _ntff_profile_via_ctypes

    mod.set_axon_ntff_profile_hook(
        _ntff_profile_via_ctypes("/opt/axon/libaxon_pjrt.so")
    )


def run(start_x, start_y, start_yaw, start_speed, accel, steering, trace=False,
        tmpdir=None):
    if "nc" not in _CACHE:
        _CACHE["nc"] = _build()
    nc = _CACHE["nc"]
    if trace:
        _install_ntff_shim()

    start_x = np.asarray(start_x, dtype=np.float32)
    start_y = np.asarray(start_y, dtype=np.float32)
    start_yaw = np.asarray(start_yaw, dtype=np.float32)
    start_speed = np.asarray(start_speed, dtype=np.float32)
    A, Bv, c = _host_precompute(np.asarray(accel), np.asarray(steering))

    in_maps = []
    for i in range(NCORES):
        sl = slice(i * BL, (i + 1) * BL)
        (rhs1, rhs2, rhsy, cdt16, lhs1, lhs2, lhsy, s0_16,
         yawp_16) = _build_mats(A, Bv, c, start_yaw[sl], start_speed[sl])
        # ACT Sin biases use the f16-QUANTIZED yaw0' so the matmul's yaw0'
        # contribution cancels exactly
        sinb = (TWO_PI * yawp_16.astype(np.float64)).astype(np.float32)
        cosb = (TWO_PI * yawp_16.astype(np.float64) + HALF_PI).astype(np.float32)
        cols = np.stack(
            [sinb, cosb, start_yaw[sl],
             (DT * start_speed[sl].astype(np.float64)).astype(np.float32),
             start_x[sl], start_y[sl], start_speed[sl]],
            axis=1,
        ).astype(np.float32)
        in_maps.append({
            "rhs1": np.ascontiguousarray(rhs1), "rhs2": np.ascontiguousarray(rhs2),
            "rhsy": np.ascontiguousarray(rhsy), "cdt16": cdt16,
            "lhs1": np.ascontiguousarray(lhs1), "lhs2": np.ascontiguousarray(lhs2),
            "lhsy": np.ascontiguousarray(lhsy),
            "cols": np.ascontiguousarray(cols),
        })

    res = run_bass_kernel_spmd(nc, in_maps, core_ids=list(range(NCORES)),
                               trace=trace, tmpdir=tmpdir)

    outs = []
    starts = (start_x, start_y, start_yaw, start_speed)
    for key, st in zip(("ox", "oy", "oyaw", "ospeed"), starts):
        full = np.concatenate(
            [res.results[i][key].astype(np.float32) for i in range(NCORES)],
            axis=0)
        out = np.ascontiguousarray(full.T)
        out[0, :] = st
        outs.append(out)
    return tuple(outs), res


def kernel(start_x, start_y, start_yaw, start_speed, accel, steering):
    outs, _ = run(start_x, start_y, start_yaw, start_speed, accel, steering)
    return outs


# revision 26
# speedup vs baseline: 1.0015x; 1.0015x over previous
"""Trainium2 Bass kernel for the kinematic bicycle-model rollout.

Strategy (v3, ~119us vs 165us baseline)
---------------------------------------
The rollout is affine in the start state with batch-independent
coefficients; the host precomputes (f64) the [H] vectors
    c[t]  = sum_{i<t} DT*MAX_ACC*clip(a_i)          (speed_t = s0 + c_t)
    A[t]  = sum_{i<t} k_i,  Bv[t] = sum_{i<t} c_i*k_i,
            k_i = tan(clip(s_i))/WHEELBASE*DT       (yaw_t = yaw0 + s0*A_t + Bv_t)
leaving x/y on-chip:  x_t = x0 + sum_{i<t} (DT*c_i + DT*s0) * cos(yaw_i).

Per core (data-parallel over batch, 1024 rollouts), batch on the 128 SBUF
partitions (8 tiles), time on the free dim.  The two DVE prefix scans are
the hard floor (~71us/core, ~2.2ns/col, dtype-independent), so everything
else is pushed off DVE:

  PE   f16 matmuls into PSUM (f16 streams ~1.2ns/col incl fp32 accum):
         nw1 = round(u) - u + yaw0'   via K=7 rows
               [yaw0', s0*A', Bv', +MAGIC, -MAGIC, -s0*A', -Bv']
               with u = yaw/(2pi); MAGIC=1.5*2^23 enters as the f16-exact
               product 1024*12288, and PSUM accumulates rows in order in
               fp32, so fl(u+MAGIC)-MAGIC realizes round(u) exactly
         nw2 = round(u+0.25) - u + yaw0'  (K=8, extra +0.25 row; the 0.25
               phase turns Sin into cos at exactly +-pi arg range)
         yawp = s0*A + Bv  (radians, K=2, 1024-wide PSUM pairs)
  ACT  sin16 = Sin(-2pi*nw1 + 2pi*yaw0'_f16)   [bias uses the f16-rounded
       cos16 = Sin(-2pi*nw2 + 2pi*yaw0'_f16 + pi/2)   yaw0' so the matmul
       term cancels exactly; args land in +-pi, LUT valid to ~+-3.2]
       yaw16 = yawp + yaw0;  m16 = cdt16 + DT*s0;  speed16 = cdt16/DT + s0
  DVE  vx16 = m16*cos16, vy16 = m16*sin16 (tensor_tensor f16, 2 elem/cyc)
       x/y prefix scans (f16 in/out, fp32 internal state, chained per
       1024-col chunk through the f16 boundary column)
GPSIMD is deliberately idle: Pool cannot run TensorScalarPtr ops or touch
PSUM on V3, and its tensor_tensor co-running with DVE inflates both ~60%
via SBUF port contention (measured 165us -> 179us).

All outputs leave the device as f16 (rel gate is 2e-2, f16 gives ~5e-4),
halving DMA traffic to 16.8MB/core; the host casts to f32, transposes to
the reference [H, B] layout, and overwrites row 0 with the exact f32
start state (device leaves x/y col 0 unwritten).
"""

import math
import sys

sys.path.insert(0, "/opt/trn_rl_repo")

import numpy as np

import concourse.bacc as bacc
import concourse.mybir as mybir
import concourse.tile as tile
from concourse.bass_utils import run_bass_kernel_spmd

# Model constants (match the reference nn.Module)
H = 2048
B = 8192
NCORES = 8
BL = B // NCORES          # batch per core
P = 128                   # SBUF partitions
NPT = BL // P             # batch tiles per core
DT = 0.05
WHEELBASE = 2.5
MAX_STEER = 0.5
MAX_ACC = 5000.0 / 1000.0

TWO_PI = 2.0 * math.pi
INV_2PI = 1.0 / TWO_PI
HALF_PI = 0.5 * math.pi
MAG_L = 1024.0            # MAGIC = MAG_L * MAG_R = 1.5*2^23, f16-exact factors
MAG_R = 12288.0

F32 = mybir.dt.float32
F16 = mybir.dt.float16
AFT = mybir.ActivationFunctionType
ALU = mybir.AluOpType

K1 = 7                    # nw1 matmul rows
K2 = 8                    # nw2 matmul rows
HH = 512                  # column segment = one PSUM bank of f32

_CACHE = {}


def _build():
    nc = bacc.Bacc("TRN2", target_bir_lowering=False, debug=False)

    rhs1 = nc.declare_dram_parameter("rhs1", [K1, H], F16, isOutput=False)
    rhs2 = nc.declare_dram_parameter("rhs2", [K2, H], F16, isOutput=False)
    rhsy = nc.declare_dram_parameter("rhsy", [2, H], F16, isOutput=False)
    lhs1 = nc.declare_dram_parameter("lhs1", [K1, BL], F16, isOutput=False)
    lhs2 = nc.declare_dram_parameter("lhs2", [K2, BL], F16, isOutput=False)
    lhsy = nc.declare_dram_parameter("lhsy", [2, BL], F16, isOutput=False)
    cdt16 = nc.declare_dram_parameter("cdt16", [H], F16, isOutput=False)
    # per-batch f32 columns: [sinb, cosb, yaw0, sdt0, x0, y0, s0]
    cols = nc.declare_dram_parameter("cols", [BL, 7], F32, isOutput=False)
    ox = nc.declare_dram_parameter("ox", [BL, H], F16, isOutput=True)
    oy = nc.declare_dram_parameter("oy", [BL, H], F16, isOutput=True)
    oyaw = nc.declare_dram_parameter("oyaw", [BL, H], F16, isOutput=True)
    ospeed = nc.declare_dram_parameter("ospeed", [BL, H], F16, isOutput=True)

    with tile.TileContext(nc) as tc:
        with (
            tc.tile_pool(name="const", bufs=1) as constp,
            tc.tile_pool(name="io", bufs=3) as iop,
            tc.tile_pool(name="mid", bufs=4) as midp,
            tc.tile_pool(name="psum", bufs=2, space="PSUM") as psp,
        ):
            cdt_bc = constp.tile([P, H], F16)
            nc.sync.dma_start(out=cdt_bc[:], in_=cdt16[None, :].to_broadcast((P, H)))
            r1_sb = constp.tile([K1, H], F16)
            nc.sync.dma_start(out=r1_sb[:], in_=rhs1[:])
            r2_sb = constp.tile([K2, H], F16)
            nc.sync.dma_start(out=r2_sb[:], in_=rhs2[:])
            ry_sb = constp.tile([2, H], F16)
            nc.sync.dma_start(out=ry_sb[:], in_=rhsy[:])
            l1_sb = constp.tile([K1, BL], F16)
            nc.sync.dma_start(out=l1_sb[:], in_=lhs1[:])
            l2_sb = constp.tile([K2, BL], F16)
            nc.sync.dma_start(out=l2_sb[:], in_=lhs2[:])
            ly_sb = constp.tile([2, BL], F16)
            nc.sync.dma_start(out=ly_sb[:], in_=lhsy[:])

            NSEG = H // HH
            for pt in range(NPT):
                sl = slice(pt * P, (pt + 1) * P)
                colt = iop.tile([P, 7], F32, tag="colt")
                nc.sync.dma_start(out=colt[:], in_=cols[sl, :])
                sinb_c = colt[:, 0:1]
                cosb_c = colt[:, 1:2]
                yaw0_c = colt[:, 2:3]
                sdt0_c = colt[:, 3:4]
                x0_c = colt[:, 4:5]
                y0_c = colt[:, 5:6]
                s0_c = colt[:, 6:7]

                # col 0 of x/y is left unwritten on device; the host
                # overwrites output row 0 with the exact f32 start state
                x_sb = iop.tile([P, H], F16, tag="x")
                y_sb = iop.tile([P, H], F16, tag="y")

                # m16/speed16 first: they only need cdt_bc+colt, so ACT can
                # emit them before the first matmul results land
                m16 = midp.tile([P, H], F16, tag="m16")
                nc.scalar.activation(out=m16[:], in_=cdt_bc[:], func=AFT.Identity,
                                     bias=sdt0_c, scale=1.0)
                speed16 = midp.tile([P, H], F16, tag="sp16")
                nc.scalar.activation(out=speed16[:], in_=cdt_bc[:],
                                     func=AFT.Identity, bias=s0_c,
                                     scale=1.0 / DT)

                sin_t = midp.tile([P, H], F16, tag="sin")
                cos_t = midp.tile([P, H], F16, tag="cos")
                yaw16 = midp.tile([P, H], F16, tag="yaw16")
                for hf in range(NSEG):
                    cs = slice(hf * HH, (hf + 1) * HH)
                    ps1 = psp.tile([P, HH], F32, tag="ps1")
                    nc.tensor.matmul(ps1[:], l1_sb[:, sl], r1_sb[:, cs])
                    ps2 = psp.tile([P, HH], F32, tag="ps2")
                    nc.tensor.matmul(ps2[:], l2_sb[:, sl], r2_sb[:, cs])
                    if hf % 2 == 0:
                        psy = psp.tile([P, 2 * HH], F32, tag="psy")
                        nc.tensor.matmul(psy[:, 0:HH], ly_sb[:, sl],
                                         ry_sb[:, cs])
                        nc.tensor.matmul(psy[:, HH:2 * HH], ly_sb[:, sl],
                                         ry_sb[:, cs.stop:cs.stop + HH])

                    nc.scalar.activation(out=cos_t[:, cs], in_=ps2[:],
                                         func=AFT.Sin, bias=cosb_c,
                                         scale=-TWO_PI)
                    nc.scalar.activation(out=sin_t[:, cs], in_=ps1[:],
                                         func=AFT.Sin, bias=sinb_c,
                                         scale=-TWO_PI)
                    if hf % 2 == 1:
                        ys = slice((hf - 1) * HH, (hf + 1) * HH)
                        nc.scalar.activation(out=yaw16[:, ys], in_=psy[:],
                                             func=AFT.Identity, bias=yaw0_c,
                                             scale=1.0)

                nc.sync.dma_start(out=oyaw[sl, :], in_=yaw16[:])
                nc.sync.dma_start(out=ospeed[sl, :], in_=speed16[:])

                vx = midp.tile([P, H], F16, tag="vx")
                vy = midp.tile([P, H], F16, tag="vy")
                # tile 0 starts with a 512 chunk so the first scan only
                # waits on one ACT segment; steady-state tiles stay coarse
                CHUNKS = ([(0, 512), (512, 512), (1024, 1024)] if pt == 0
                          else [(0, 1024), (1024, 1024)])
                for ci, (lo, sz) in enumerate(CHUNKS):
                    bs = slice(lo, lo + sz)
                    nd = sz if ci < len(CHUNKS) - 1 else sz - 1
                    nc.vector.tensor_tensor(out=vx[:, bs], in0=m16[:, bs],
                                            in1=cos_t[:, bs], op=ALU.mult)
                    nc.vector.tensor_tensor_scan(
                        out=x_sb[:, lo + 1:lo + 1 + nd], data0=vx[:, lo:lo + nd],
                        data1=vx[:, lo:lo + nd],
                        initial=(x0_c if lo == 0 else x_sb[:, lo:lo + 1]),
                        op0=ALU.add, op1=ALU.bypass,
                    )
                    nc.vector.tensor_tensor(out=vy[:, bs], in0=m16[:, bs],
                                            in1=sin_t[:, bs], op=ALU.mult)
                    nc.vector.tensor_tensor_scan(
                        out=y_sb[:, lo + 1:lo + 1 + nd], data0=vy[:, lo:lo + nd],
                        data1=vy[:, lo:lo + nd],
                        initial=(y0_c if lo == 0 else y_sb[:, lo:lo + 1]),
                        op0=ALU.add, op1=ALU.bypass,
                    )
                    nc.sync.dma_start(out=ox[sl, bs], in_=x_sb[:, bs])
                    nc.sync.dma_start(out=oy[sl, bs], in_=y_sb[:, bs])

    nc.finalize()
    return nc


def _host_precompute(accel, steering):
    a = np.clip(accel.astype(np.float64), -1.0, 1.0)
    dv = DT * MAX_ACC * a
    c = np.concatenate([[0.0], np.cumsum(dv)[: H - 1]])
    st = np.clip(steering.astype(np.float64), -MAX_STEER, MAX_STEER)
    k = np.tan(st) / WHEELBASE * DT
    A = np.concatenate([[0.0], np.cumsum(k)[: H - 1]])
    Bv = np.concatenate([[0.0], np.cumsum(c * k)[: H - 1]])
    return A, Bv, c


def _build_mats(A, Bv, c, start_yaw, start_speed):
    """Host-side f16 matmul operands + f32 bias columns (per core slice)."""
    Ap = (A * INV_2PI).astype(np.float16)
    Bp = (Bv * INV_2PI).astype(np.float16)
    ones_h = np.ones(H, np.float16)
    rhs1 = np.stack([ones_h, Ap, Bp, ones_h * MAG_R, ones_h * MAG_R, Ap, Bp])
    rhs2 = np.stack([ones_h, Ap, Bp, ones_h * 0.25, ones_h * MAG_R,
                     ones_h * MAG_R, Ap, Bp])
    rhsy = np.stack([A.astype(np.float16), Bv.astype(np.float16)])
    cdt16 = (DT * c).astype(np.float16)

    s0_16 = start_speed.astype(np.float16)
    yawp_16 = (start_yaw.astype(np.float64) * INV_2PI).astype(np.float16)
    ones_b = np.ones(BL, np.float16)
    lhs1 = np.stack([yawp_16, s0_16, ones_b, ones_b * MAG_L, -ones_b * MAG_L,
                     -s0_16, -ones_b])
    lhs2 = np.stack([yawp_16, s0_16, ones_b, ones_b, ones_b * MAG_L,
                     -ones_b * MAG_L, -s0_16, -ones_b])
    lhsy = np.stack([s0_16, ones_b])
    return rhs1, rhs2, rhsy, cdt16, lhs1, lhs2, lhsy, s0_16, yawp_16


def _install_ntff_shim():
    """antenv.axon_hooks is absent in this image; recreate it so
    run_bass_kernel_spmd(trace=True) can reach the axon NTFF profiler."""
    import types

    import antenv

    if hasattr(antenv, "axon_hooks"):
        return
    mod = types.ModuleType("antenv.axon_hooks")
    holder = [None]
    mod.set_axon_ntff_profile_hook = lambda h: holder.__setitem__(0, h)
    mod.get_axon_ntff_profile_hook = lambda: holder[0]
    sys.modules["antenv.axon_hooks"] = mod
    antenv.axon_hooks = mod
    from trn_agent_boot.trn_boot import _ntff_profile_via_ctypes

    mod.set_axon_ntff_profile_hook(
        _ntff_profile_via_ctypes("/opt/axon/libaxon_pjrt.so")
    )


def run(start_x, start_y, start_yaw, start_speed, accel, steering, trace=False,
        tmpdir=None):
    if "nc" not in _CACHE:
        _CACHE["nc"] = _build()
    nc = _CACHE["nc"]
    if trace:
        _install_ntff_shim()

    start_x = np.asarray(start_x, dtype=np.float32)
    start_y = np.asarray(start_y, dtype=np.float32)
    start_yaw = np.asarray(start_yaw, dtype=np.float32)
    start_speed = np.asarray(start_speed, dtype=np.float32)
    A, Bv, c = _host_precompute(np.asarray(accel), np.asarray(steering))

    in_maps = []
    for i in range(NCORES):
        sl = slice(i * BL, (i + 1) * BL)
        (rhs1, rhs2, rhsy, cdt16, lhs1, lhs2, lhsy, s0_16,
         yawp_16) = _build_mats(A, Bv, c, start_yaw[sl], start_speed[sl])
        # ACT Sin biases use the f16-QUANTIZED yaw0' so the matmul's yaw0'
        # contribution cancels exactly
        sinb = (TWO_PI * yawp_16.astype(np.float64)).astype(np.float32)
        cosb = (TWO_PI * yawp_16.astype(np.float64) + HALF_PI).astype(np.float32)
        cols = np.stack(
            [sinb, cosb, start_yaw[sl],
             (DT * start_speed[sl].astype(np.float64)).astype(np.float32),
             start_x[sl], start_y[sl], start_speed[sl]],
            axis=1,
        ).astype(np.float32)
        in_maps.append({
            "rhs1": np.ascontiguousarray(rhs1), "rhs2": np.ascontiguousarray(rhs2),
            "rhsy": np.ascontiguousarray(rhsy), "cdt16": cdt16,
            "lhs1": np.ascontiguousarray(lhs1), "lhs2": np.ascontiguousarray(lhs2),
            "lhsy": np.ascontiguousarray(lhsy),
            "cols": np.ascontiguousarray(cols),
        })

    res = run_bass_kernel_spmd(nc, in_maps, core_ids=list(range(NCORES)),
                               trace=trace, tmpdir=tmpdir)

    outs = []
    starts = (start_x, start_y, start_yaw, start_speed)
    for key, st in zip(("ox", "oy", "oyaw", "ospeed"), starts):
        full = np.concatenate(
            [res.results[i][key].astype(np.float32) for i in range(NCORES)],
            axis=0)
        out = np.ascontiguousarray(full.T)
        out[0, :] = st
        outs.append(out)
    return tuple(outs), res


def kernel(start_x, start_y, start_yaw, start_speed, accel, steering):
    outs, _ = run(start_x, start_y, start_yaw, start_speed, accel, steering)
    return outs


# revision 28
# speedup vs baseline: 1.0163x; 1.0147x over previous
"""Trainium2 Bass kernel for the kinematic bicycle-model rollout.

Strategy (v3, ~118us vs 165us baseline)
---------------------------------------
The rollout is affine in the start state with batch-independent
coefficients; the host precomputes (f64) the [H] vectors
    c[t]  = sum_{i<t} DT*MAX_ACC*clip(a_i)          (speed_t = s0 + c_t)
    A[t]  = sum_{i<t} k_i,  Bv[t] = sum_{i<t} c_i*k_i,
            k_i = tan(clip(s_i))/WHEELBASE*DT       (yaw_t = yaw0 + s0*A_t + Bv_t)
leaving x/y on-chip:  x_t = x0 + sum_{i<t} (DT*c_i + DT*s0) * cos(yaw_i).

Per core (data-parallel over batch, 1024 rollouts), batch on the 128 SBUF
partitions (8 tiles), time on the free dim.  The two DVE prefix scans are
the hard floor (~71us/core, ~2.2ns/col, dtype-independent), so everything
else is pushed off DVE:

  PE   f16 matmuls into PSUM (f16 streams ~1.2ns/col incl fp32 accum):
         nw1 = round(u) - u + yaw0'   via K=7 rows
               [yaw0', s0*A', Bv', +MAGIC, -MAGIC, -s0*A', -Bv']
               with u = yaw/(2pi); MAGIC=1.5*2^23 enters as the f16-exact
               product 1024*12288, and PSUM accumulates rows in order in
               fp32, so fl(u+MAGIC)-MAGIC realizes round(u) exactly
         nw2 = round(u+0.25) - u + yaw0'  (K=8, extra +0.25 row; the 0.25
               phase turns Sin into cos at exactly +-pi arg range)
         yawp = s0*A + Bv  (radians, K=2, 1024-wide PSUM pairs)
  ACT  sin16 = Sin(-2pi*nw1 + 2pi*yaw0'_f16)   [bias uses the f16-rounded
       cos16 = Sin(-2pi*nw2 + 2pi*yaw0'_f16 + pi/2)   yaw0' so the matmul
       term cancels exactly; args land in +-pi, LUT valid to ~+-3.2]
       yaw16 = yawp + yaw0;  m16 = cdt16 + DT*s0;  speed16 = cdt16/DT + s0
  DVE  vx16 = m16*cos16, vy16 = m16*sin16 (tensor_tensor f16, 2 elem/cyc)
       x/y prefix scans (f16 in/out, fp32 internal state, chained per
       1024-col chunk through the f16 boundary column; tile 0 uses a 512
       first chunk to shorten pipeline fill, and cos is emitted before
       sin each segment since vx/scan-x are DVE's first consumers)
GPSIMD is deliberately idle: Pool cannot run TensorScalarPtr ops or touch
PSUM on V3, and its tensor_tensor co-running with DVE inflates both ~60%
via SBUF port contention (measured 165us -> 179us).

All outputs leave the device as f16 (rel gate is 2e-2, f16 gives ~5e-4),
halving DMA traffic to 16.8MB/core; the host casts to f32, transposes to
the reference [H, B] layout, and overwrites row 0 with the exact f32
start state (device leaves x/y col 0 unwritten).
"""

import math
import sys

sys.path.insert(0, "/opt/trn_rl_repo")

import numpy as np

import concourse.bacc as bacc
import concourse.mybir as mybir
import concourse.tile as tile
from concourse.bass_utils import run_bass_kernel_spmd

# Model constants (match the reference nn.Module)
H = 2048
B = 8192
NCORES = 8
BL = B // NCORES          # batch per core
P = 128                   # SBUF partitions
NPT = BL // P             # batch tiles per core
DT = 0.05
WHEELBASE = 2.5
MAX_STEER = 0.5
MAX_ACC = 5000.0 / 1000.0

TWO_PI = 2.0 * math.pi
INV_2PI = 1.0 / TWO_PI
HALF_PI = 0.5 * math.pi
MAG_L = 1024.0            # MAGIC = MAG_L * MAG_R = 1.5*2^23, f16-exact factors
MAG_R = 12288.0

F32 = mybir.dt.float32
F16 = mybir.dt.float16
AFT = mybir.ActivationFunctionType
ALU = mybir.AluOpType

K1 = 7                    # nw1 matmul rows
K2 = 8                    # nw2 matmul rows
HH = 512                  # column segment = one PSUM bank of f32

_CACHE = {}


def _build():
    nc = bacc.Bacc("TRN2", target_bir_lowering=False, debug=False)

    rhs1 = nc.declare_dram_parameter("rhs1", [K1, H], F16, isOutput=False)
    rhs2 = nc.declare_dram_parameter("rhs2", [K2, H], F16, isOutput=False)
    rhsy = nc.declare_dram_parameter("rhsy", [2, H], F16, isOutput=False)
    lhs1 = nc.declare_dram_parameter("lhs1", [K1, BL], F16, isOutput=False)
    lhs2 = nc.declare_dram_parameter("lhs2", [K2, BL], F16, isOutput=False)
    lhsy = nc.declare_dram_parameter("lhsy", [2, BL], F16, isOutput=False)
    cdt16 = nc.declare_dram_parameter("cdt16", [H], F16, isOutput=False)
    # per-batch f32 columns: [sinb, cosb, yaw0, sdt0, x0, y0, s0]
    cols = nc.declare_dram_parameter("cols", [BL, 7], F32, isOutput=False)
    ox = nc.declare_dram_parameter("ox", [BL, H], F16, isOutput=True)
    oy = nc.declare_dram_parameter("oy", [BL, H], F16, isOutput=True)
    oyaw = nc.declare_dram_parameter("oyaw", [BL, H], F16, isOutput=True)
    ospeed = nc.declare_dram_parameter("ospeed", [BL, H], F16, isOutput=True)

    with tile.TileContext(nc) as tc:
        with (
            tc.tile_pool(name="const", bufs=1) as constp,
            tc.tile_pool(name="io", bufs=3) as iop,
            tc.tile_pool(name="mid", bufs=4) as midp,
            tc.tile_pool(name="psum", bufs=2, space="PSUM") as psp,
        ):
            cdt_bc = constp.tile([P, H], F16)
            nc.sync.dma_start(out=cdt_bc[:], in_=cdt16[None, :].to_broadcast((P, H)))
            r1_sb = constp.tile([K1, H], F16)
            nc.sync.dma_start(out=r1_sb[:], in_=rhs1[:])
            r2_sb = constp.tile([K2, H], F16)
            nc.sync.dma_start(out=r2_sb[:], in_=rhs2[:])
            ry_sb = constp.tile([2, H], F16)
            nc.sync.dma_start(out=ry_sb[:], in_=rhsy[:])
            l1_sb = constp.tile([K1, BL], F16)
            nc.sync.dma_start(out=l1_sb[:], in_=lhs1[:])
            l2_sb = constp.tile([K2, BL], F16)
            nc.sync.dma_start(out=l2_sb[:], in_=lhs2[:])
            ly_sb = constp.tile([2, BL], F16)
            nc.sync.dma_start(out=ly_sb[:], in_=lhsy[:])

            NSEG = H // HH
            for pt in range(NPT):
                sl = slice(pt * P, (pt + 1) * P)
                colt = iop.tile([P, 7], F32, tag="colt")
                nc.sync.dma_start(out=colt[:], in_=cols[sl, :])
                sinb_c = colt[:, 0:1]
                cosb_c = colt[:, 1:2]
                yaw0_c = colt[:, 2:3]
                sdt0_c = colt[:, 3:4]
                x0_c = colt[:, 4:5]
                y0_c = colt[:, 5:6]
                s0_c = colt[:, 6:7]

                # col 0 of x/y is left unwritten on device; the host
                # overwrites output row 0 with the exact f32 start state
                x_sb = iop.tile([P, H], F16, tag="x")
                y_sb = iop.tile([P, H], F16, tag="y")

                # m16/speed16 first: they only need cdt_bc+colt, so ACT can
                # emit them before the first matmul results land
                m16 = midp.tile([P, H], F16, tag="m16")
                nc.scalar.activation(out=m16[:], in_=cdt_bc[:], func=AFT.Identity,
                                     bias=sdt0_c, scale=1.0)
                speed16 = midp.tile([P, H], F16, tag="sp16")
                nc.scalar.activation(out=speed16[:], in_=cdt_bc[:],
                                     func=AFT.Identity, bias=s0_c,
                                     scale=1.0 / DT)

                sin_t = midp.tile([P, H], F16, tag="sin")
                cos_t = midp.tile([P, H], F16, tag="cos")
                yaw16 = midp.tile([P, H], F16, tag="yaw16")
                for hf in range(NSEG):
                    cs = slice(hf * HH, (hf + 1) * HH)
                    ps1 = psp.tile([P, HH], F32, tag="ps1")
                    nc.tensor.matmul(ps1[:], l1_sb[:, sl], r1_sb[:, cs])
                    ps2 = psp.tile([P, HH], F32, tag="ps2")
                    nc.tensor.matmul(ps2[:], l2_sb[:, sl], r2_sb[:, cs])
                    if hf % 2 == 0:
                        psy = psp.tile([P, 2 * HH], F32, tag="psy")
                        nc.tensor.matmul(psy[:, 0:HH], ly_sb[:, sl],
                                         ry_sb[:, cs])
                        nc.tensor.matmul(psy[:, HH:2 * HH], ly_sb[:, sl],
                                         ry_sb[:, cs.stop:cs.stop + HH])

                    nc.scalar.activation(out=cos_t[:, cs], in_=ps2[:],
                                         func=AFT.Sin, bias=cosb_c,
                                         scale=-TWO_PI)
                    nc.scalar.activation(out=sin_t[:, cs], in_=ps1[:],
                                         func=AFT.Sin, bias=sinb_c,
                                         scale=-TWO_PI)
                    if hf % 2 == 1:
                        ys = slice((hf - 1) * HH, (hf + 1) * HH)
                        nc.scalar.activation(out=yaw16[:, ys], in_=psy[:],
                                             func=AFT.Identity, bias=yaw0_c,
                                             scale=1.0)

                nc.sync.dma_start(out=oyaw[sl, :], in_=yaw16[:])
                nc.sync.dma_start(out=ospeed[sl, :], in_=speed16[:])

                vx = midp.tile([P, H], F16, tag="vx")
                vy = midp.tile([P, H], F16, tag="vy")
                # tile 0 starts with a 512 chunk so the first scan only
                # waits on one ACT segment; steady-state tiles stay coarse
                # pair-scan: state = (vx_even + state) + vx_odd walks two
                # timesteps per scan column, writing x at even t in 1023 cols;
                # odd t then fills via one strided tt: x_{2j+1} = x_{2j}+vx_{2j}
                for (v, o_sb, init_c, odram) in (
                    (vx, x_sb, x0_c, ox), (vy, y_sb, y0_c, oy)
                ):
                    src = cos_t if v is vx else sin_t
                    nc.vector.tensor_tensor(out=v[:], in0=m16[:], in1=src[:],
                                            op=ALU.mult)
                    nc.vector.tensor_tensor_scan(
                        out=o_sb[:, 2:2048:2], data0=v[:, 0:2046:2],
                        data1=v[:, 1:2047:2], initial=init_c,
                        op0=ALU.add, op1=ALU.add,
                    )
                    nc.scalar.activation(out=o_sb[:, 1:2], in_=v[:, 0:1],
                                         func=AFT.Identity, bias=init_c,
                                         scale=1.0)
                    nc.vector.tensor_tensor(out=o_sb[:, 3:2048:2],
                                            in0=o_sb[:, 2:2047:2],
                                            in1=v[:, 2:2047:2], op=ALU.add)
                    nc.sync.dma_start(out=odram[sl, :], in_=o_sb[:])

    nc.finalize()
    return nc


def _host_precompute(accel, steering):
    a = np.clip(accel.astype(np.float64), -1.0, 1.0)
    dv = DT * MAX_ACC * a
    c = np.concatenate([[0.0], np.cumsum(dv)[: H - 1]])
    st = np.clip(steering.astype(np.float64), -MAX_STEER, MAX_STEER)
    k = np.tan(st) / WHEELBASE * DT
    A = np.concatenate([[0.0], np.cumsum(k)[: H - 1]])
    Bv = np.concatenate([[0.0], np.cumsum(c * k)[: H - 1]])
    return A, Bv, c


def _build_mats(A, Bv, c, start_yaw, start_speed):
    """Host-side f16 matmul operands + f32 bias columns (per core slice)."""
    Ap = (A * INV_2PI).astype(np.float16)
    Bp = (Bv * INV_2PI).astype(np.float16)
    ones_h = np.ones(H, np.float16)
    rhs1 = np.stack([ones_h, Ap, Bp, ones_h * MAG_R, ones_h * MAG_R, Ap, Bp])
    rhs2 = np.stack([ones_h, Ap, Bp, ones_h * 0.25, ones_h * MAG_R,
                     ones_h * MAG_R, Ap, Bp])
    rhsy = np.stack([A.astype(np.float16), Bv.astype(np.float16)])
    cdt16 = (DT * c).astype(np.float16)

    s0_16 = start_speed.astype(np.float16)
    yawp_16 = (start_yaw.astype(np.float64) * INV_2PI).astype(np.float16)
    ones_b = np.ones(BL, np.float16)
    lhs1 = np.stack([yawp_16, s0_16, ones_b, ones_b * MAG_L, -ones_b * MAG_L,
                     -s0_16, -ones_b])
    lhs2 = np.stack([yawp_16, s0_16, ones_b, ones_b, ones_b * MAG_L,
                     -ones_b * MAG_L, -s0_16, -ones_b])
    lhsy = np.stack([s0_16, ones_b])
    return rhs1, rhs2, rhsy, cdt16, lhs1, lhs2, lhsy, s0_16, yawp_16


def _install_ntff_shim():
    """antenv.axon_hooks is absent in this image; recreate it so
    run_bass_kernel_spmd(trace=True) can reach the axon NTFF profiler."""
    import types

    import antenv

    if hasattr(antenv, "axon_hooks"):
        return
    mod = types.ModuleType("antenv.axon_hooks")
    holder = [None]
    mod.set_axon_ntff_profile_hook = lambda h: holder.__setitem__(0, h)
    mod.get_axon_ntff_profile_hook = lambda: holder[0]
    sys.modules["antenv.axon_hooks"] = mod
    antenv.axon_hooks = mod
    from trn_agent_boot.trn_boot import _ntff_profile_via_ctypes

    mod.set_axon_ntff_profile_hook(
        _ntff_profile_via_ctypes("/opt/axon/libaxon_pjrt.so")
    )


def run(start_x, start_y, start_yaw, start_speed, accel, steering, trace=False,
        tmpdir=None):
    if "nc" not in _CACHE:
        _CACHE["nc"] = _build()
    nc = _CACHE["nc"]
    if trace:
        _install_ntff_shim()

    start_x = np.asarray(start_x, dtype=np.float32)
    start_y = np.asarray(start_y, dtype=np.float32)
    start_yaw = np.asarray(start_yaw, dtype=np.float32)
    start_speed = np.asarray(start_speed, dtype=np.float32)
    A, Bv, c = _host_precompute(np.asarray(accel), np.asarray(steering))

    in_maps = []
    for i in range(NCORES):
        sl = slice(i * BL, (i + 1) * BL)
        (rhs1, rhs2, rhsy, cdt16, lhs1, lhs2, lhsy, s0_16,
         yawp_16) = _build_mats(A, Bv, c, start_yaw[sl], start_speed[sl])
        # ACT Sin biases use the f16-QUANTIZED yaw0' so the matmul's yaw0'
        # contribution cancels exactly
        sinb = (TWO_PI * yawp_16.astype(np.float64)).astype(np.float32)
        cosb = (TWO_PI * yawp_16.astype(np.float64) + HALF_PI).astype(np.float32)
        cols = np.stack(
            [sinb, cosb, start_yaw[sl],
             (DT * start_speed[sl].astype(np.float64)).astype(np.float32),
             start_x[sl], start_y[sl], start_speed[sl]],
            axis=1,
        ).astype(np.float32)
        in_maps.append({
            "rhs1": np.ascontiguousarray(rhs1), "rhs2": np.ascontiguousarray(rhs2),
            "rhsy": np.ascontiguousarray(rhsy), "cdt16": cdt16,
            "lhs1": np.ascontiguousarray(lhs1), "lhs2": np.ascontiguousarray(lhs2),
            "lhsy": np.ascontiguousarray(lhsy),
            "cols": np.ascontiguousarray(cols),
        })

    res = run_bass_kernel_spmd(nc, in_maps, core_ids=list(range(NCORES)),
                               trace=trace, tmpdir=tmpdir)

    outs = []
    starts = (start_x, start_y, start_yaw, start_speed)
    for key, st in zip(("ox", "oy", "oyaw", "ospeed"), starts):
        full = np.concatenate(
            [res.results[i][key].astype(np.float32) for i in range(NCORES)],
            axis=0)
        out = np.ascontiguousarray(full.T)
        out[0, :] = st
        outs.append(out)
    return tuple(outs), res


def kernel(start_x, start_y, start_yaw, start_speed, accel, steering):
    outs, _ = run(start_x, start_y, start_yaw, start_speed, accel, steering)
    return outs


# revision 29
# speedup vs baseline: 1.1070x; 1.0893x over previous
"""Trainium2 Bass kernel for the kinematic bicycle-model rollout.

Strategy (v3, ~118us vs 165us baseline)
---------------------------------------
The rollout is affine in the start state with batch-independent
coefficients; the host precomputes (f64) the [H] vectors
    c[t]  = sum_{i<t} DT*MAX_ACC*clip(a_i)          (speed_t = s0 + c_t)
    A[t]  = sum_{i<t} k_i,  Bv[t] = sum_{i<t} c_i*k_i,
            k_i = tan(clip(s_i))/WHEELBASE*DT       (yaw_t = yaw0 + s0*A_t + Bv_t)
leaving x/y on-chip:  x_t = x0 + sum_{i<t} (DT*c_i + DT*s0) * cos(yaw_i).

Per core (data-parallel over batch, 1024 rollouts), batch on the 128 SBUF
partitions (8 tiles), time on the free dim.  The two DVE prefix scans are
the hard floor (~71us/core, ~2.2ns/col, dtype-independent), so everything
else is pushed off DVE:

  PE   f16 matmuls into PSUM (f16 streams ~1.2ns/col incl fp32 accum):
         nw1 = round(u) - u + yaw0'   via K=7 rows
               [yaw0', s0*A', Bv', +MAGIC, -MAGIC, -s0*A', -Bv']
               with u = yaw/(2pi); MAGIC=1.5*2^23 enters as the f16-exact
               product 1024*12288, and PSUM accumulates rows in order in
               fp32, so fl(u+MAGIC)-MAGIC realizes round(u) exactly
         nw2 = round(u+0.25) - u + yaw0'  (K=8, extra +0.25 row; the 0.25
               phase turns Sin into cos at exactly +-pi arg range)
         yawp = s0*A + Bv  (radians, K=2, 1024-wide PSUM pairs)
  ACT  sin16 = Sin(-2pi*nw1 + 2pi*yaw0'_f16)   [bias uses the f16-rounded
       cos16 = Sin(-2pi*nw2 + 2pi*yaw0'_f16 + pi/2)   yaw0' so the matmul
       term cancels exactly; args land in +-pi, LUT valid to ~+-3.2]
       yaw16 = yawp + yaw0;  m16 = cdt16 + DT*s0;  speed16 = cdt16/DT + s0
  DVE  vx16 = m16*cos16, vy16 = m16*sin16 (tensor_tensor f16, 2 elem/cyc)
       x/y prefix scans (f16 in/out, fp32 internal state, chained per
       1024-col chunk through the f16 boundary column; tile 0 uses a 512
       first chunk to shorten pipeline fill, and cos is emitted before
       sin each segment since vx/scan-x are DVE's first consumers)
GPSIMD is deliberately idle: Pool cannot run TensorScalarPtr ops or touch
PSUM on V3, and its tensor_tensor co-running with DVE inflates both ~60%
via SBUF port contention (measured 165us -> 179us).

All outputs leave the device as f16 (rel gate is 2e-2, f16 gives ~5e-4),
halving DMA traffic to 16.8MB/core; the host casts to f32, transposes to
the reference [H, B] layout, and overwrites row 0 with the exact f32
start state (device leaves x/y col 0 unwritten).
"""

import math
import sys

sys.path.insert(0, "/opt/trn_rl_repo")

import numpy as np

import concourse.bacc as bacc
import concourse.mybir as mybir
import concourse.tile as tile
from concourse.bass_utils import run_bass_kernel_spmd

# Model constants (match the reference nn.Module)
H = 2048
B = 8192
NCORES = 8
BL = B // NCORES          # batch per core
P = 128                   # SBUF partitions
NPT = BL // P             # batch tiles per core
DT = 0.05
WHEELBASE = 2.5
MAX_STEER = 0.5
MAX_ACC = 5000.0 / 1000.0

TWO_PI = 2.0 * math.pi
INV_2PI = 1.0 / TWO_PI
HALF_PI = 0.5 * math.pi
MAG_L = 1024.0            # MAGIC = MAG_L * MAG_R = 1.5*2^23, f16-exact factors
MAG_R = 12288.0

F32 = mybir.dt.float32
F16 = mybir.dt.float16
AFT = mybir.ActivationFunctionType
ALU = mybir.AluOpType

K1 = 7                    # nw1 matmul rows
K2 = 8                    # nw2 matmul rows
HH = 512                  # column segment = one PSUM bank of f32

_CACHE = {}


def _build():
    nc = bacc.Bacc("TRN2", target_bir_lowering=False, debug=False)

    rhs1 = nc.declare_dram_parameter("rhs1", [K1, H], F16, isOutput=False)
    rhs2 = nc.declare_dram_parameter("rhs2", [K2, H], F16, isOutput=False)
    rhsy = nc.declare_dram_parameter("rhsy", [2, H], F16, isOutput=False)
    lhs1 = nc.declare_dram_parameter("lhs1", [K1, BL], F16, isOutput=False)
    lhs2 = nc.declare_dram_parameter("lhs2", [K2, BL], F16, isOutput=False)
    lhsy = nc.declare_dram_parameter("lhsy", [2, BL], F16, isOutput=False)
    cdt16 = nc.declare_dram_parameter("cdt16", [H], F16, isOutput=False)
    # per-batch f32 columns: [sinb, cosb, yaw0, sdt0, x0, y0, s0]
    cols = nc.declare_dram_parameter("cols", [BL, 7], F32, isOutput=False)
    ox = nc.declare_dram_parameter("ox", [BL, H], F16, isOutput=True)
    oy = nc.declare_dram_parameter("oy", [BL, H], F16, isOutput=True)
    oyaw = nc.declare_dram_parameter("oyaw", [BL, H], F16, isOutput=True)
    ospeed = nc.declare_dram_parameter("ospeed", [BL, H], F16, isOutput=True)

    with tile.TileContext(nc) as tc:
        with (
            tc.tile_pool(name="const", bufs=1) as constp,
            tc.tile_pool(name="io", bufs=3) as iop,
            tc.tile_pool(name="mid", bufs=4) as midp,
            tc.tile_pool(name="psum", bufs=2, space="PSUM") as psp,
        ):
            cdt_bc = constp.tile([P, H], F16)
            nc.sync.dma_start(out=cdt_bc[:], in_=cdt16[None, :].to_broadcast((P, H)))
            r1_sb = constp.tile([K1, H], F16)
            nc.sync.dma_start(out=r1_sb[:], in_=rhs1[:])
            r2_sb = constp.tile([K2, H], F16)
            nc.sync.dma_start(out=r2_sb[:], in_=rhs2[:])
            ry_sb = constp.tile([2, H], F16)
            nc.sync.dma_start(out=ry_sb[:], in_=rhsy[:])
            l1_sb = constp.tile([K1, BL], F16)
            nc.sync.dma_start(out=l1_sb[:], in_=lhs1[:])
            l2_sb = constp.tile([K2, BL], F16)
            nc.sync.dma_start(out=l2_sb[:], in_=lhs2[:])
            ly_sb = constp.tile([2, BL], F16)
            nc.sync.dma_start(out=ly_sb[:], in_=lhsy[:])

            NSEG = H // HH
            for pt in range(NPT):
                sl = slice(pt * P, (pt + 1) * P)
                colt = iop.tile([P, 7], F32, tag="colt")
                nc.sync.dma_start(out=colt[:], in_=cols[sl, :])
                sinb_c = colt[:, 0:1]
                cosb_c = colt[:, 1:2]
                yaw0_c = colt[:, 2:3]
                sdt0_c = colt[:, 3:4]
                x0_c = colt[:, 4:5]
                y0_c = colt[:, 5:6]
                s0_c = colt[:, 6:7]

                # col 0 of x/y is left unwritten on device; the host
                # overwrites output row 0 with the exact f32 start state
                x_sb = iop.tile([P, H], F16, tag="x")
                y_sb = iop.tile([P, H], F16, tag="y")

                # m16/speed16 first: they only need cdt_bc+colt, so ACT can
                # emit them before the first matmul results land
                m16 = midp.tile([P, H], F16, tag="m16")
                nc.scalar.activation(out=m16[:], in_=cdt_bc[:], func=AFT.Identity,
                                     bias=sdt0_c, scale=1.0)
                speed16 = midp.tile([P, H], F16, tag="sp16")
                nc.vector.tensor_scalar(out=speed16[:], in0=cdt_bc[:],
                                        scalar1=1.0 / DT, scalar2=s0_c,
                                        op0=ALU.mult, op1=ALU.add)

                sin_t = midp.tile([P, H], F16, tag="sin")
                cos_t = midp.tile([P, H], F16, tag="cos")
                yaw16 = midp.tile([P, H], F16, tag="yaw16")
                for hf in range(NSEG):
                    cs = slice(hf * HH, (hf + 1) * HH)
                    ps1 = psp.tile([P, HH], F32, tag="ps1")
                    nc.tensor.matmul(ps1[:], l1_sb[:, sl], r1_sb[:, cs])
                    ps2 = psp.tile([P, HH], F32, tag="ps2")
                    nc.tensor.matmul(ps2[:], l2_sb[:, sl], r2_sb[:, cs])
                    if hf % 2 == 0:
                        psy = psp.tile([P, 2 * HH], F32, tag="psy")
                        nc.tensor.matmul(psy[:, 0:HH], ly_sb[:, sl],
                                         ry_sb[:, cs])
                        nc.tensor.matmul(psy[:, HH:2 * HH], ly_sb[:, sl],
                                         ry_sb[:, cs.stop:cs.stop + HH])

                    nc.scalar.activation(out=cos_t[:, cs], in_=ps2[:],
                                         func=AFT.Sin, bias=cosb_c,
                                         scale=-TWO_PI)
                    nc.scalar.activation(out=sin_t[:, cs], in_=ps1[:],
                                         func=AFT.Sin, bias=sinb_c,
                                         scale=-TWO_PI)
                    if hf % 2 == 1:
                        ys = slice((hf - 1) * HH, (hf + 1) * HH)
                        nc.scalar.activation(out=yaw16[:, ys], in_=psy[:],
                                             func=AFT.Identity, bias=yaw0_c,
                                             scale=1.0)

                nc.sync.dma_start(out=oyaw[sl, :], in_=yaw16[:])
                nc.sync.dma_start(out=ospeed[sl, :], in_=speed16[:])

                vx = midp.tile([P, H], F16, tag="vx")
                vy = midp.tile([P, H], F16, tag="vy")
                # tile 0 starts with a 512 chunk so the first scan only
                # waits on one ACT segment; steady-state tiles stay coarse
                # pair-scan: state = (vx_even + state) + vx_odd walks two
                # timesteps per scan column, writing x at even t in 1023 cols;
                # odd t then fills via one strided tt: x_{2j+1} = x_{2j}+vx_{2j}
                for (v, o_sb, init_c, odram) in (
                    (vx, x_sb, x0_c, ox), (vy, y_sb, y0_c, oy)
                ):
                    src = cos_t if v is vx else sin_t
                    nc.vector.tensor_tensor(out=v[:], in0=m16[:], in1=src[:],
                                            op=ALU.mult)
                    nc.vector.tensor_tensor_scan(
                        out=o_sb[:, 2:2048:2], data0=v[:, 0:2046:2],
                        data1=v[:, 1:2047:2], initial=init_c,
                        op0=ALU.add, op1=ALU.add,
                    )
                    nc.scalar.activation(out=o_sb[:, 1:2], in_=v[:, 0:1],
                                         func=AFT.Identity, bias=init_c,
                                         scale=1.0)
                    nc.vector.tensor_tensor(out=o_sb[:, 3:2048:2],
                                            in0=o_sb[:, 2:2047:2],
                                            in1=v[:, 2:2047:2], op=ALU.add)
                    nc.sync.dma_start(out=odram[sl, :], in_=o_sb[:])

    nc.finalize()
    return nc


def _host_precompute(accel, steering):
    a = np.clip(accel.astype(np.float64), -1.0, 1.0)
    dv = DT * MAX_ACC * a
    c = np.concatenate([[0.0], np.cumsum(dv)[: H - 1]])
    st = np.clip(steering.astype(np.float64), -MAX_STEER, MAX_STEER)
    k = np.tan(st) / WHEELBASE * DT
    A = np.concatenate([[0.0], np.cumsum(k)[: H - 1]])
    Bv = np.concatenate([[0.0], np.cumsum(c * k)[: H - 1]])
    return A, Bv, c


def _build_mats(A, Bv, c, start_yaw, start_speed):
    """Host-side f16 matmul operands + f32 bias columns (per core slice)."""
    Ap = (A * INV_2PI).astype(np.float16)
    Bp = (Bv * INV_2PI).astype(np.float16)
    ones_h = np.ones(H, np.float16)
    rhs1 = np.stack([ones_h, Ap, Bp, ones_h * MAG_R, ones_h * MAG_R, Ap, Bp])
    rhs2 = np.stack([ones_h, Ap, Bp, ones_h * 0.25, ones_h * MAG_R,
                     ones_h * MAG_R, Ap, Bp])
    rhsy = np.stack([A.astype(np.float16), Bv.astype(np.float16)])
    cdt16 = (DT * c).astype(np.float16)

    s0_16 = start_speed.astype(np.float16)
    yawp_16 = (start_yaw.astype(np.float64) * INV_2PI).astype(np.float16)
    ones_b = np.ones(BL, np.float16)
    lhs1 = np.stack([yawp_16, s0_16, ones_b, ones_b * MAG_L, -ones_b * MAG_L,
                     -s0_16, -ones_b])
    lhs2 = np.stack([yawp_16, s0_16, ones_b, ones_b, ones_b * MAG_L,
                     -ones_b * MAG_L, -s0_16, -ones_b])
    lhsy = np.stack([s0_16, ones_b])
    return rhs1, rhs2, rhsy, cdt16, lhs1, lhs2, lhsy, s0_16, yawp_16


def _install_ntff_shim():
    """antenv.axon_hooks is absent in this image; recreate it so
    run_bass_kernel_spmd(trace=True) can reach the axon NTFF profiler."""
    import types

    import antenv

    if hasattr(antenv, "axon_hooks"):
        return
    mod = types.ModuleType("antenv.axon_hooks")
    holder = [None]
    mod.set_axon_ntff_profile_hook = lambda h: holder.__setitem__(0, h)
    mod.get_axon_ntff_profile_hook = lambda: holder[0]
    sys.modules["antenv.axon_hooks"] = mod
    antenv.axon_hooks = mod
    from trn_agent_boot.trn_boot import _ntff_profile_via_ctypes

    mod.set_axon_ntff_profile_hook(
        _ntff_profile_via_ctypes("/opt/axon/libaxon_pjrt.so")
    )


def run(start_x, start_y, start_yaw, start_speed, accel, steering, trace=False,
        tmpdir=None):
    if "nc" not in _CACHE:
        _CACHE["nc"] = _build()
    nc = _CACHE["nc"]
    if trace:
        _install_ntff_shim()

    start_x = np.asarray(start_x, dtype=np.float32)
    start_y = np.asarray(start_y, dtype=np.float32)
    start_yaw = np.asarray(start_yaw, dtype=np.float32)
    start_speed = np.asarray(start_speed, dtype=np.float32)
    A, Bv, c = _host_precompute(np.asarray(accel), np.asarray(steering))

    in_maps = []
    for i in range(NCORES):
        sl = slice(i * BL, (i + 1) * BL)
        (rhs1, rhs2, rhsy, cdt16, lhs1, lhs2, lhsy, s0_16,
         yawp_16) = _build_mats(A, Bv, c, start_yaw[sl], start_speed[sl])
        # ACT Sin biases use the f16-QUANTIZED yaw0' so the matmul's yaw0'
        # contribution cancels exactly
        sinb = (TWO_PI * yawp_16.astype(np.float64)).astype(np.float32)
        cosb = (TWO_PI * yawp_16.astype(np.float64) + HALF_PI).astype(np.float32)
        cols = np.stack(
            [sinb, cosb, start_yaw[sl],
             (DT * start_speed[sl].astype(np.float64)).astype(np.float32),
             start_x[sl], start_y[sl], start_speed[sl]],
            axis=1,
        ).astype(np.float32)
        in_maps.append({
            "rhs1": np.ascontiguousarray(rhs1), "rhs2": np.ascontiguousarray(rhs2),
            "rhsy": np.ascontiguousarray(rhsy), "cdt16": cdt16,
            "lhs1": np.ascontiguousarray(lhs1), "lhs2": np.ascontiguousarray(lhs2),
            "lhsy": np.ascontiguousarray(lhsy),
            "cols": np.ascontiguousarray(cols),
        })

    res = run_bass_kernel_spmd(nc, in_maps, core_ids=list(range(NCORES)),
                               trace=trace, tmpdir=tmpdir)

    outs = []
    starts = (start_x, start_y, start_yaw, start_speed)
    for key, st in zip(("ox", "oy", "oyaw", "ospeed"), starts):
        full = np.concatenate(
            [res.results[i][key].astype(np.float32) for i in range(NCORES)],
            axis=0)
        out = np.ascontiguousarray(full.T)
        out[0, :] = st
        outs.append(out)
    return tuple(outs), res


def kernel(start_x, start_y, start_yaw, start_speed, accel, steering):
    outs, _ = run(start_x, start_y, start_yaw, start_speed, accel, steering)
    return outs
